# revision 1
# baseline (speedup 1.0000x reference)
"""Bass/Trainium2 kernel for the 2-layer GAT (nn_GAT_11106785427688).

Strategy (8 NeuronCores, SPMD single NEFF):
- dst-ownership sharding: core c owns nodes [c*OWN, (c+1)*OWN); it receives
  every edge whose dst it owns (~137K edges), so segment-softmax denominators
  and message sums complete locally -- no all-reduce. One AllGather of the
  layer-1 activations between layers; host assembles the final output from
  per-core slices.
- Per-edge gather of packed [h | a_src.h] rows (fp16, 256B) from an HBM table
  via the SWDGE dma_gather custom op (int16 indices -> src buckets of 32768
  rows; table rows permuted so the dense phase writes 2KB-contiguous runs).
- No indexed scatter (HW dma_scatter_add loses duplicate updates): edges are
  grouped by 128-node dst window; one-hot R [edges x nodes] (fp16) built on
  DVE via iota-compare turns segment-sum into PE matmul accumulated in PSUM.
  Softmax division is deferred: out = (sum_e w*h[src]) / (sum_e w).
- exp(leakyrelu(e)) computed without max-subtraction (shift-invariant).
"""
import numpy as np
import ml_dtypes

from concourse import bacc, mybir
import concourse.tile as tile
from concourse.bass_utils import run_bass_kernel_spmd

# ---- problem constants ----
N = 100000
D = 64
H1, C1 = 4, 16
NEG = 0.2
NCORES = 8
OWN = 12544                 # 98 windows * 128 per core
BUCK = 32768
CHUNK = 1024                # gather idxs per dma_gather call (ring limit)
TPC = CHUNK // 128          # tiles per chunk = 8

F16 = mybir.dt.float16
F32 = mybir.dt.float32
BF16 = mybir.dt.bfloat16
I16 = mybir.dt.int16
NPF16 = np.float16
NPBF16 = ml_dtypes.bfloat16


def _derived():
    NW = OWN // 128
    NPAD = NCORES * OWN
    NBUCK = (NPAD + BUCK - 1) // BUCK
    TBL_ROWS = NBUCK * BUCK
    return NW, NPAD, NBUCK, TBL_ROWS


def _perm_row(src):
    """Permuted table row for node src: tb*1024 + p*8 + j (write-friendly)."""
    tb, r = np.divmod(src, 1024)
    j, p = np.divmod(r, 128)
    return tb * 1024 + p * 8 + j


def prep(edge_index):
    """Vectorized host prep. Returns (sched, idx_h, dcol_h, drow_h)."""
    NW, NPAD, NBUCK, TBL_ROWS = _derived()
    src = np.concatenate([np.asarray(edge_index[0]), np.arange(N, dtype=np.int64)])
    dst = np.concatenate([np.asarray(edge_index[1]), np.arange(N, dtype=np.int64)])
    owner = dst // OWN

    # balanced bucket width (1024-aligned for the table-row permutation)
    BS = ((NPAD + NBUCK - 1) // NBUCK + 1023) // 1024 * 1024
    assert BS <= BUCK
    per_core = []
    counts = np.zeros((NCORES, NBUCK * NW), np.int64)
    for c in range(NCORES):
        m = owner == c
        s = src[m]
        d = dst[m] - c * OWN
        w = d >> 7
        b = s // BS
        key = b * NW + w
        order = np.lexsort((s, key))
        s, d, key = s[order], d[order], key[order]
        per_core.append((s, d, key))
        counts[c] = np.bincount(key, minlength=NBUCK * NW)

    gsize = ((counts.max(0) + 127) // 128 * 128).astype(np.int64)  # [NBUCK*NW]
    gs2 = gsize.reshape(NBUCK, NW)
    slots_b = gs2.sum(1)
    slots_bp = (slots_b + CHUNK - 1) // CHUNK * CHUNK
    bucket_base = np.concatenate([[0], np.cumsum(slots_bp)])
    total_slots = int(bucket_base[-1])
    n_chunks = total_slots // CHUNK

    # group slot starts within the global layout
    gstart = np.zeros(NBUCK * NW, np.int64)
    for bb in range(NBUCK):
        gstart[bb * NW:(bb + 1) * NW] = bucket_base[bb] + np.concatenate(
            [[0], np.cumsum(gs2[bb][:-1])])

    # tile schedule
    n_tiles = total_slots // 128
    tile_w = np.full(n_tiles, -1, np.int64)
    tile_first = np.zeros(n_tiles, bool)
    tile_last = np.zeros(n_tiles, bool)
    for g in range(NBUCK * NW):
        t0 = gstart[g] // 128
        nt = gsize[g] // 128
        if nt == 0:
            continue
        tile_w[t0:t0 + nt] = g % NW
        tile_first[t0] = True
        tile_last[t0 + nt - 1] = True
    chunk_bucket = np.zeros(n_chunks, np.int64)
    for bb in range(NBUCK):
        chunk_bucket[bucket_base[bb] // CHUNK: bucket_base[bb + 1] // CHUNK] = bb

    # per-core slot arrays (vectorized)
    idx_h = np.zeros((NCORES, 128, n_chunks * (CHUNK // 16)), np.int16)
    dcol_h = np.zeros((NCORES, 128, n_chunks * TPC), NPF16)
    drow_h = np.zeros((NCORES, n_chunks, 1, CHUNK), NPBF16)
    for c in range(NCORES):
        s, d, key = per_core[c]
        grp_first = np.searchsorted(key, np.arange(NBUCK * NW))
        rank = np.arange(len(s)) - grp_first[key]
        slot = gstart[key] + rank
        gi = np.zeros(total_slots, np.int64)
        off = np.full(total_slots, -1, np.int64)
        pr = _perm_row(s)
        gi[slot] = pr - (s // BS) * BS
        off[slot] = d & 127
        assert (gi >= 0).all() and (gi < BS).all()
        # idx wrap: slot i of chunk ch at [i%16 + 16k, ch*64 + i//16]
        gia = gi.reshape(n_chunks, CHUNK // 16, 16).transpose(0, 2, 1)  # [ch,16,64]
        idx_h[c] = np.tile(gia, (1, 8, 1)).transpose(1, 0, 2).reshape(128, -1)
        # dcol: [p, ch*8+j] = off(ch*1024 + j*128 + p)
        offa = off.reshape(n_chunks, TPC, 128).transpose(2, 0, 1).reshape(128, -1)
        dcol_h[c] = offa.astype(NPF16)
        drow_h[c] = off.reshape(n_chunks, 1, CHUNK).astype(NPBF16)

    sched = dict(n_chunks=n_chunks, tile_w=tile_w.tolist(),
                 tile_first=tile_first.tolist(), tile_last=tile_last.tolist(),
                 chunk_bucket=chunk_bucket.tolist(), bs=BS)
    return sched, idx_h, dcol_h, drow_h


def build(sched, debug=False, no_collective=False):
    NW, NPAD, NBUCK, TBL_ROWS = _derived()
    n_chunks = sched["n_chunks"]
    tile_w = sched["tile_w"]
    tile_first = sched["tile_first"]
    tile_last = sched["tile_last"]
    chunk_bucket = sched["chunk_bucket"]
    BS = sched["bs"]
    NT_DENSE = NPAD // 128
    NB_DENSE = (NT_DENSE + 7) // 8

    nc = bacc.Bacc(None, target_bir_lowering=False, num_swdge_queues=4)

    embT = nc.dram_tensor("embT", [D, NPAD], F32, kind="ExternalInput")
    embTo = nc.dram_tensor("embTo", [D, OWN], F32, kind="ExternalInput")
    w1aux = nc.dram_tensor("w1aux", [D, D + H1], F32, kind="ExternalInput")
    w1ad = nc.dram_tensor("w1ad", [D, H1], F32, kind="ExternalInput")
    w2aux = nc.dram_tensor("w2aux", [D, D + 1], F32, kind="ExternalInput")
    w2ad = nc.dram_tensor("w2ad", [D, 1], F32, kind="ExternalInput")
    b1t_in = nc.dram_tensor("b1t", [128, D], F32, kind="ExternalInput")
    b2t_in = nc.dram_tensor("b2t", [128, D], F32, kind="ExternalInput")
    iota_in = nc.dram_tensor("iotac", [128, 128], F16, kind="ExternalInput")
    pconst_in = nc.dram_tensor("pconst", [128, 1], F32, kind="ExternalInput")
    ident_in = nc.dram_tensor("ident", [128, 128], F32, kind="ExternalInput")
    ones_in = nc.dram_tensor("ones1", [1, 128], BF16, kind="ExternalInput")
    idx_in = nc.dram_tensor("idx16", [128, n_chunks * (CHUNK // 16)], I16, kind="ExternalInput")
    dcol_in = nc.dram_tensor("dcol", [128, n_chunks * TPC], F16, kind="ExternalInput")
    drow_in = nc.dram_tensor("drow", [n_chunks, 1, CHUNK], BF16, kind="ExternalInput")
    out_own = nc.dram_tensor("out_own", [OWN, D], F32, kind="ExternalOutput")

    if debug:
        dbg_acc1 = nc.dram_tensor("dbg_acc1", [128, NW * (D + H1)], F32,
                                  kind="ExternalOutput")
        dbg_tbl = nc.dram_tensor("dbg_tbl", [TBL_ROWS, 128], F16,
                                 kind="ExternalOutput")
    table = nc.dram_tensor("table", [TBL_ROWS, 128], F16)
    ag_in = nc.dram_tensor("ag_in", [D, OWN], F32)
    ag_out = nc.dram_tensor("ag_out", [NCORES * D, OWN], F32, addr_space="Shared")

    with tile.TileContext(nc) as tc:
        with tc.tile_pool(name="persist", bufs=1) as pp:
            b1t = pp.tile([128, D], F32)
            b2t = pp.tile([128, D], F32)
            iotac = pp.tile([128, 128], F16)
            pconst = pp.tile([128, 1], F32)
            ident = pp.tile([128, 128], F32)
            ones1 = pp.tile([1, 128], BF16)
            w1x = pp.tile([D, D + H1], F32)
            w1d = pp.tile([D, H1], F32)
            w2x = pp.tile([D, D + 1], F32)
            w2d = pp.tile([D, 1], F32)
            idx_s = pp.tile([128, n_chunks * (CHUNK // 16)], I16)
            dcol_s = pp.tile([128, n_chunks * TPC], F16)
            adw = pp.tile([128, NW * H1], F16)
            adw2 = pp.tile([128, NW], F16)
            acc1 = pp.tile([128, NW * (D + H1)], F32)
            acc2 = pp.tile([128, NW * (D + 1)], F32)
            for t_, s_ in [(b1t, b1t_in), (b2t, b2t_in), (iotac, iota_in),
                           (pconst, pconst_in), (ident, ident_in), (ones1, ones_in),
                           (w1x, w1aux), (w1d, w1ad), (w2x, w2aux), (w2d, w2ad),
                           (idx_s, idx_in), (dcol_s, dcol_in)]:
                nc.sync.dma_start(out=t_[:], in_=s_[:])
            nc.vector.memset(acc1[:], 0.0)
            nc.vector.memset(acc2[:], 0.0)

            def dense(layer):
                """x @ Waux -> fp16 table rows (permuted layout)."""
                waux = w1x if layer == 1 else w2x
                ncol = D + H1 if layer == 1 else D + 1
                with tc.tile_pool(name=f"dns{layer}", bufs=3) as dp, \
                     tc.tile_pool(name=f"dnp{layer}", bufs=3, space="PSUM") as dpp:
                    for tb in range(NB_DENSE):
                        lt = dp.tile([D, 1024], F32, tag="lhs")
                        if layer == 1:
                            nc.sync.dma_start(out=lt[:], in_=embT[:, tb * 1024:(tb + 1) * 1024])
                        else:
                            # global tiles 8tb..8tb+7 -> (core, window) runs
                            j = 0
                            while j < 8:
                                t = tb * 8 + j
                                co, wl = divmod(t, NW)
                                nrun = min(8 - j, NW - wl)
                                nc.sync.dma_start(
                                    out=lt[:, j * 128:(j + nrun) * 128],
                                    in_=ag_out[co * D:(co + 1) * D,
                                               wl * 128:(wl + nrun) * 128])
                                j += nrun
                        stg = dp.tile([128, 8 * 128], F16, tag="stg")
                        nc.vector.memset(stg[:], 0.0)
                        for j in range(8):
                            ps = dpp.tile([128, ncol], F32, tag="d")
                            nc.tensor.matmul(out=ps[:], lhsT=lt[:, j * 128:(j + 1) * 128],
                                             rhs=waux[:], start=True, stop=True)
                            nc.vector.tensor_copy(out=stg[:, j * 128:j * 128 + ncol],
                                                  in_=ps[:])
                        nc.sync.dma_start(
                            out=table[tb * 1024:(tb + 1) * 1024].rearrange(
                                "(p j) k -> p (j k)", j=8),
                            in_=stg[:])

            def adw_fill(layer):
                """Per-owned-window a_dst.h via x_own @ (W @ Ad)."""
                wad = w1d if layer == 1 else w2d
                H = H1 if layer == 1 else 1
                dst_t = adw if layer == 1 else adw2
                srcT = embTo if layer == 1 else ag_in
                with tc.tile_pool(name=f"aw{layer}", bufs=3) as ap, \
                     tc.tile_pool(name=f"awp{layer}", bufs=3, space="PSUM") as app:
                    for w in range(NW):
                        lt = ap.tile([D, 128], F32, tag="l")
                        nc.sync.dma_start(out=lt[:], in_=srcT[:, w * 128:(w + 1) * 128])
                        ps = app.tile([128, H], F32, tag="p")
                        nc.tensor.matmul(out=ps[:], lhsT=lt[:], rhs=wad[:],
                                         start=True, stop=True)
                        nc.vector.tensor_copy(out=dst_t[:, w * H:(w + 1) * H], in_=ps[:])

            def edge_sweep(layer):
                H = H1 if layer == 1 else 1
                CH = C1 if layer == 1 else D
                EC = D + H
                acc = acc1 if layer == 1 else acc2
                adwl = adw if layer == 1 else adw2
                with tc.tile_pool(name=f"eg{layer}", bufs=6) as gp, \
                     tc.tile_pool(name=f"er{layer}", bufs=4) as rp, \
                     tc.tile_pool(name=f"em{layer}", bufs=6) as mp, \
                     tc.tile_pool(name=f"epr{layer}", bufs=2, space="PSUM") as prp, \
                     tc.tile_pool(name=f"epa{layer}", bufs=2, space="PSUM") as pap, \
                     tc.tile_pool(name=f"epg{layer}", bufs=2, space="PSUM") as pgp:
                    group_ps = {}
                    for c in range(n_chunks):
                        bb = chunk_bucket[c]
                        live = [j for j in range(TPC) if tile_w[c * TPC + j] >= 0]
                        assert live == list(range(len(live))), "pads must trail"
                        nl = len(live)
                        ght = gp.tile([128, TPC * 128], F16, tag="ght")
                        nc.gpsimd.dma_gather(
                            ght[:].rearrange("p (a k) -> p a k", k=128),
                            table[bb * BS:bb * BS + BS, :],
                            idx_s[:, c * (CHUNK // 16):(c + 1) * (CHUNK // 16)],
                            CHUNK, CHUNK, 128, elem_step=128, queue_num=c % 4)
                        if not live:
                            continue
                        ght3 = ght[:].rearrange("p (a k) -> p a k", k=128)
                        # replicate dstoff row via K=1 matmuls (bf16)
                        drt = gp.tile([1, CHUNK], BF16, tag="drow")
                        nc.sync.dma_start(out=drt[:], in_=drow_in[c])
                        psr = prp.tile([128, CHUNK], F32, tag="r")
                        for hh in range(2):
                            nc.tensor.matmul(
                                out=psr[:, hh * 512:(hh + 1) * 512],
                                lhsT=ones1[:],
                                rhs=drt[0:1, hh * 512:(hh + 1) * 512],
                                start=True, stop=True)
                        R = rp.tile([128, TPC * 128], F16, tag="R")
                        nc.vector.tensor_tensor(
                            out=R[:].rearrange("p (a k) -> p a k", k=128),
                            in0=dcol_s[:, c * TPC:(c + 1) * TPC, None].to_broadcast(
                                [128, TPC, 128]),
                            in1=iotac[:, None, :].to_broadcast([128, TPC, 128]),
                            op=mybir.AluOpType.is_equal)
                        RT = rp.tile([128, TPC * 128], F16, tag="RT")
                        nc.vector.tensor_tensor(
                            out=RT[:],
                            in0=pconst[:].to_broadcast([128, TPC * 128]),
                            in1=psr[:],
                            op=mybir.AluOpType.is_equal)
                        psa = pap.tile([128, nl * H], F32, tag="a", name=f"psa{c}")
                        for j in live:
                            w = tile_w[c * TPC + j]
                            nc.tensor.matmul(
                                out=psa[:, j * H:(j + 1) * H],
                                lhsT=RT[:, j * 128:(j + 1) * 128],
                                rhs=adwl[:, w * H:(w + 1) * H],
                                start=True, stop=True)
                        ew = mp.tile([128, nl * H], F32, tag="ew", name=f"ew{c}")
                        nc.vector.tensor_tensor(
                            out=ew[:].rearrange("p (a h) -> p a h", h=H),
                            in0=psa[:].rearrange("p (a h) -> p a h", h=H),
                            in1=ght3[:, 0:nl, D:D + H],
                            op=mybir.AluOpType.add)
                        lr = mp.tile([128, nl * H], F32, tag="lr", name=f"lr{c}")
                        nc.vector.tensor_scalar_mul(out=lr[:], in0=ew[:], scalar1=NEG)
                        nc.vector.tensor_tensor(out=lr[:], in0=lr[:], in1=ew[:],
                                                op=mybir.AluOpType.max)
                        msgt = mp.tile([128, nl * EC], F16, tag="msg", name=f"msg{c}")
                        msgt3 = msgt[:].rearrange("p (a k) -> p a k", k=EC)
                        nc.scalar.activation(
                            out=msgt3[:, :, D:D + H],
                            in_=lr[:].rearrange("p (a h) -> p a h", h=H),
                            func=mybir.ActivationFunctionType.Exp)
                        nc.vector.tensor_tensor(
                            out=msgt3[:, :, 0:D].rearrange("p a (h k) -> p a h k", k=CH),
                            in0=ght3[:, 0:nl, 0:D].rearrange("p a (h k) -> p a h k", k=CH),
                            in1=msgt3[:, :, D:D + H, None].to_broadcast(
                                [128, nl, H, CH]),
                            op=mybir.AluOpType.mult)
                        for j in live:
                            t = c * TPC + j
                            w = tile_w[t]
                            if tile_first[t]:
                                group_ps[w] = pgp.tile([128, EC], F32, tag="g", name=f"grp{w}")
                            ps = group_ps[w]
                            nc.tensor.matmul(
                                out=ps[:], lhsT=R[:, j * 128:(j + 1) * 128],
                                rhs=msgt[:, j * EC:(j + 1) * EC],
                                start=tile_first[t], stop=tile_last[t])
                            if tile_last[t]:
                                nc.vector.tensor_tensor(
                                    out=acc[:, w * EC:(w + 1) * EC],
                                    in0=acc[:, w * EC:(w + 1) * EC],
                                    in1=ps[:], op=mybir.AluOpType.add)
                                del group_ps[w]
                    assert not group_ps

            # ================= layer 1 =================
            dense(1)
            adw_fill(1)
            if debug:
                with tc.tile_pool(name="dbgt", bufs=2) as dtp:
                    for tb in range(NB_DENSE):
                        t_ = dtp.tile([128, 8 * 128], F16, tag="d")
                        nc.sync.dma_start(
                            out=t_[:],
                            in_=table[tb * 1024:(tb + 1) * 1024].rearrange(
                                "(p j) k -> p (j k)", j=8))
                        nc.sync.dma_start(
                            out=dbg_tbl[tb * 1024:(tb + 1) * 1024].rearrange(
                                "(p j) k -> p (j k)", j=8),
                            in_=t_[:])
            edge_sweep(1)
            if debug:
                nc.sync.dma_start(out=dbg_acc1[:], in_=acc1[:])
            with tc.tile_pool(name="f1", bufs=3) as fp, \
                 tc.tile_pool(name="f1p", bufs=3, space="PSUM") as fpp:
                EC = D + H1
                for w in range(NW):
                    den = fp.tile([128, H1], F32, tag="den")
                    nc.vector.tensor_scalar_add(
                        out=den[:], in0=acc1[:, w * EC + D:(w + 1) * EC], scalar1=1e-16)
                    rec = fp.tile([128, H1], F32, tag="rec")
                    nc.vector.reciprocal(out=rec[:], in_=den[:])
                    x2 = fp.tile([128, D], F32, tag="x2")
                    nc.vector.tensor_tensor(
                        out=x2[:].rearrange("p (h k) -> p h k", k=C1),
                        in0=acc1[:, w * EC:w * EC + D].rearrange("p (h k) -> p h k", k=C1),
                        in1=rec[:, :, None].to_broadcast([128, H1, C1]),
                        op=mybir.AluOpType.mult)
                    nc.vector.tensor_tensor(out=x2[:], in0=x2[:], in1=b1t[:],
                                            op=mybir.AluOpType.add)
                    # elu(x) = relu(x) - relu(1 - exp(x))
                    ex = fp.tile([128, D], F32, tag="ex")
                    nc.scalar.activation(out=ex[:], in_=x2[:],
                                         func=mybir.ActivationFunctionType.Exp)
                    u = fp.tile([128, D], F32, tag="u")
                    nc.scalar.activation(out=u[:], in_=ex[:],
                                         func=mybir.ActivationFunctionType.Relu,
                                         scale=-1.0, bias=1.0)
                    r = fp.tile([128, D], F32, tag="r")
                    nc.scalar.activation(out=r[:], in_=x2[:],
                                         func=mybir.ActivationFunctionType.Relu)
                    xe = fp.tile([128, D], F32, tag="xe")
                    nc.vector.tensor_tensor(out=xe[:], in0=r[:], in1=u[:],
                                            op=mybir.AluOpType.subtract)
                    pst = fpp.tile([D, 128], F32, tag="t")
                    nc.tensor.transpose(out=pst[:], in_=xe[:], identity=ident[:])
                    xt = fp.tile([D, 128], F32, tag="xt")
                    nc.vector.tensor_copy(out=xt[:], in_=pst[:])
                    nc.sync.dma_start(out=ag_in[:, w * 128:(w + 1) * 128], in_=xt[:])

            if no_collective:
                # profiling-only variant: local copy stands in for AllGather
                with tc.tile_pool(name="agcp", bufs=2) as acp:
                    for cc in range(NCORES):
                        t_ = acp.tile([D, OWN], F32, tag="agc")
                        nc.sync.dma_start(out=t_[:], in_=ag_in[:])
                        nc.sync.dma_start(out=ag_out[cc * D:(cc + 1) * D, :], in_=t_[:])
            else:
                nc.gpsimd.collective_compute(
                    "AllGather", mybir.AluOpType.bypass,
                    ins=[ag_in[:]], outs=[ag_out[:]],
                    replica_groups=[list(range(NCORES))])

            # ================= layer 2 =================
            dense(2)
            adw_fill(2)
            edge_sweep(2)
            with tc.tile_pool(name="f2", bufs=3) as fp:
                EC = D + 1
                for w in range(NW):
                    den = fp.tile([128, 1], F32, tag="den")
                    nc.vector.tensor_scalar_add(
                        out=den[:], in0=acc2[:, w * EC + D:(w + 1) * EC], scalar1=1e-16)
                    rec = fp.tile([128, 1], F32, tag="rec")
                    nc.vector.reciprocal(out=rec[:], in_=den[:])
                    o = fp.tile([128, D], F32, tag="o")
                    nc.vector.tensor_tensor(
                        out=o[:], in0=acc2[:, w * EC:w * EC + D],
                        in1=rec[:].to_broadcast([128, D]), op=mybir.AluOpType.mult)
                    nc.vector.tensor_tensor(out=o[:], in0=o[:], in1=b2t[:],
                                            op=mybir.AluOpType.add)
                    sq = fp.tile([128, D], F32, tag="sq")
                    ss = fp.tile([128, 1], F32, tag="ss")
                    nc.scalar.activation(out=sq[:], in_=o[:],
                                         func=mybir.ActivationFunctionType.Square,
                                         accum_out=ss[:])
                    nrm = fp.tile([128, 1], F32, tag="nr")
                    nc.scalar.activation(out=nrm[:], in_=ss[:],
                                         func=mybir.ActivationFunctionType.Sqrt)
                    nc.vector.tensor_scalar_max(out=nrm[:], in0=nrm[:], scalar1=1e-12)
                    rn = fp.tile([128, 1], F32, tag="rn")
                    nc.vector.reciprocal(out=rn[:], in_=nrm[:])
                    of = fp.tile([128, D], F32, tag="of")
                    nc.vector.tensor_tensor(out=of[:], in0=o[:],
                                            in1=rn[:].to_broadcast([128, D]),
                                            op=mybir.AluOpType.mult)
                    nc.sync.dma_start(out=out_own[w * 128:(w + 1) * 128, :], in_=of[:])

    return nc


def make_inputs(edge_index, emb, W1, a_src1, a_dst1, b1, W2, a_src2, a_dst2, b2):
    NW, NPAD, NBUCK, TBL_ROWS = _derived()
    sched, idx_h, dcol_h, drow_h = prep(edge_index)

    W1 = np.asarray(W1, np.float32)
    a_s1 = np.asarray(a_src1, np.float32)
    a_d1 = np.asarray(a_dst1, np.float32)
    As = np.zeros((D, H1), np.float32)
    Ad = np.zeros((D, H1), np.float32)
    for h in range(H1):
        As[h * C1:(h + 1) * C1, h] = a_s1[h]
        Ad[h * C1:(h + 1) * C1, h] = a_d1[h]
    w1x = np.concatenate([W1, W1 @ As], 1).astype(np.float32)
    w1d = (W1 @ Ad).astype(np.float32)
    W2 = np.asarray(W2, np.float32)
    w2x = np.concatenate([W2, W2 @ np.asarray(a_src2, np.float32).T], 1).astype(np.float32)
    w2d = (W2 @ np.asarray(a_dst2, np.float32).T).astype(np.float32)

    embT = np.zeros((D, NPAD), np.float32)
    embT[:, :N] = np.asarray(emb, np.float32).T
    iotac = np.broadcast_to(np.arange(128, dtype=NPF16)[None, :], (128, 128)).copy()
    pconst = np.arange(128, dtype=np.float32)[:, None].copy()
    ident = np.eye(128, dtype=np.float32)
    ones1 = np.ones((1, 128), NPBF16)
    b1t = np.broadcast_to(np.asarray(b1, np.float32)[None, :], (128, D)).copy()
    b2t = np.broadcast_to(np.asarray(b2, np.float32)[None, :], (128, D)).copy()

    in_maps = []
    for c in range(NCORES):
        in_maps.append({
            "embT": embT, "embTo": np.ascontiguousarray(embT[:, c * OWN:(c + 1) * OWN]),
            "w1aux": w1x, "w1ad": w1d, "w2aux": w2x, "w2ad": w2d,
            "b1t": b1t, "b2t": b2t, "iotac": iotac, "pconst": pconst,
            "ident": ident, "ones1": ones1,
            "idx16": idx_h[c], "dcol": dcol_h[c], "drow": drow_h[c],
        })
    return sched, in_maps


def kernel(edge_index, emb, W1, a_src1, a_dst1, b1, W2, a_src2, a_dst2, b2):
    sched, in_maps = make_inputs(edge_index, emb, W1, a_src1, a_dst1, b1,
                                 W2, a_src2, a_dst2, b2)
    nc = build(sched)
    nc.finalize()
    res = run_bass_kernel_spmd(nc, in_maps, core_ids=list(range(NCORES)))
    out = np.zeros((N, D), np.float32)
    for c in range(NCORES):
        lo, hi = c * OWN, min((c + 1) * OWN, N)
        if lo < N:
            out[lo:hi] = res.results[c]["out_own"][:hi - lo]
    return out

